# revision 1
# baseline (speedup 1.0000x reference)
"""Trainium2 Bass kernel for nn_Diffuser (sparse_attention).

Key algebraic identity: the reference attention has NO softmax, so
    y_rh = (q k_rh^T s)(q k_rh^T s)^T v = s^2 * q (k_rh^T k_rh) (q^T v)
    mean_r y_rh = q @ Gbar_h @ (q_h^T v_h),
    Gbar_h = s^2/R * sum_r k_rh^T k_rh   (64x64, precomputed once).

Per step, per head: w = q^T v (64x64), P = Gbar w (64x64), z^T = P^T-as-lhsT
@ q^T.  The O(N^3) attention chain disappears entirely.

Sharding: pure data-parallel over B=8 across 8 NeuronCores (weights + ref
replicated, zero collectives).  All matmuls contract over the partition dim
(c-major layouts); f32r (TF32-like) full 128x128 tiles only — sub-tile shapes
are zero-padded (f32r rejects PE row/col tiling).
"""

import numpy as np
from contextlib import ExitStack

import concourse.bass as bass
import concourse.tile as tile
from concourse import bacc, mybir
from concourse.bass_utils import run_bass_kernel_spmd
from concourse.masks import make_identity
from concourse.tile_rust import add_dep_helper

F32 = mybir.dt.float32
F32R = mybir.dt.float32r
AF = mybir.ActivationFunctionType

D = 768
H = 12
HD = 64
R = 10
N = 256
STEPS = 3
NB = 8
CC = D // 128
SCALE = HD ** -0.5
EPS = 1e-5
GS = SCALE * SCALE / R  # folded into Gbar


def _emit(nc, tc, ctx, t_x, t_ref, t_Wqv, t_Wk, t_Wproj, t_bproj, t_gamma, t_beta, t_out,
          iters=1):
    const = ctx.enter_context(tc.tile_pool(name="const", bufs=1))
    persist = ctx.enter_context(tc.tile_pool(name="persist", bufs=1))

    ident = const.tile([128, 128], F32)
    make_identity(nc, ident)
    ones_f = const.tile([128, 128], F32)
    nc.vector.memset(ones_f, 1.0)
    eps_sb = const.tile([128, 1], F32)
    nc.vector.memset(eps_sb, EPS)
    ones128 = const.tile([128, 128], F32R)
    nc.scalar.copy(ones128[:], ones_f[:])
    zsrc = const.tile([128, N], F32)
    nc.vector.memset(zsrc, 0.0)

    gamma_sb = const.tile([128, CC], F32)
    beta_sb = const.tile([128, CC], F32)
    bproj_sb = const.tile([128, CC], F32)
    nc.sync.dma_start(out=gamma_sb, in_=t_gamma.ap().rearrange("(c p) -> p c", p=128))
    nc.sync.dma_start(out=beta_sb, in_=t_beta.ap().rearrange("(c p) -> p c", p=128))
    nc.sync.dma_start(out=bproj_sb, in_=t_bproj.ap().rearrange("(c p) -> p c", p=128))

    # ---- resident weights ----
    Wproj_r = []
    with tc.tile_pool(name="wload", bufs=3) as wload:
        for cc in range(CC):
            w = wload.tile([128, D], F32, tag="wproj")
            nc.sync.dma_start(out=w, in_=t_Wproj.ap()[cc * 128:(cc + 1) * 128, :])
            wr = persist.tile([128, D], F32R, tag=f"wprojr{cc}")
            nc.vector.tensor_copy(wr[:], w[:])
            Wproj_r.append(wr)

    xT = [persist.tile([128, N], F32R, tag=f"xT{cc}", name=f"xT{cc}") for cc in range(CC)]
    qT = [persist.tile([128, N], F32R, tag=f"qT{cc}", name=f"qT{cc}") for cc in range(CC)]
    v_pad = [persist.tile([128, H * 128], F32R, tag=f"vp{p}", name=f"vp{p}")
             for p in range(2)]
    qn = [persist.tile([128, 2 * 128], F32R, tag=f"qn{h}", name=f"qn{h}") for h in range(H)]
    G_sb = [persist.tile([128, 128], F32R, tag=f"G{h}", name=f"G{h}") for h in range(H)]
    Pz = [persist.tile([128, 128], F32R, tag=f"Pz{h}", name=f"Pz{h}") for h in range(H)]
    qg_sb = [persist.tile([128, N], F32R, tag=f"qg{h}", name=f"qg{h}") for h in range(H)]
    m_sb = persist.tile([128, H * N], F32, tag="m_sb")
    zT = [persist.tile([128, N], F32R, tag=f"zT{cc}", name=f"zT{cc}") for cc in range(CC)]
    xp_sb = [persist.tile([128, N], F32R, tag=f"xp{cc}", name=f"xp{cc}") for cc in range(CC)]
    sq_sb = [persist.tile([128, N], F32R, tag=f"sq{cc}", name=f"sq{cc}") for cc in range(CC)]

    # zero-fill pads once (via ACT so f32r consumers see a rounding producer)
    for h in range(H):
        par = h % 2
        for pc in range(2):
            nc.scalar.activation(
                v_pad[pc][:, h * 128 + (1 - par) * 64: h * 128 + (2 - par) * 64],
                zsrc[:, 0:64], AF.Copy, scale=0.0)
        for nch in range(2):
            nc.scalar.activation(
                qn[h][:, nch * 128 + (1 - par) * 64: nch * 128 + (2 - par) * 64],
                zsrc[:, 0:64], AF.Copy, scale=0.0)
        nc.scalar.activation(G_sb[h][:], zsrc[:, 0:128], AF.Copy, scale=0.0)
        nc.scalar.activation(Pz[h][:, (1 - par) * 64:(2 - par) * 64],
                             zsrc[:, 0:64], AF.Copy, scale=0.0)

    def one_pass(it):
        # ---- x -> xT (c-major) ----
        with tc.tile_pool(name=f"xload{it}", bufs=2) as xload, \
             tc.tile_pool(name=f"tps{it}", bufs=3, space="PSUM") as tps:
            for nch in range(2):
                xn = xload.tile([128, D], F32, tag="xn")
                nc.sync.dma_start(out=xn, in_=t_x.ap()[nch * 128:(nch + 1) * 128, :])
                for cc in range(CC):
                    pt = tps.tile([128, 128], F32, tag="pt")
                    nc.tensor.transpose(pt[:], xn[:, cc * 128:(cc + 1) * 128], ident[:])
                    nc.vector.tensor_copy(xT[cc][:, nch * 128:(nch + 1) * 128], pt[:])

        # ---- Gbar_h = s^2/R * Wk_h^T (ref^T ref) Wk_h ----
        # S = ref^T ref contracts over ref's natural partition dim: NO
        # transposes.  S accumulated in two PSUM passes (8 + 4 banks) over
        # streamed ref chunks; then U = S @ Wk, Gbar_h = Wk_h^T U_h via the
        # sliding head-pair lhsT trick.
        with tc.tile_pool(name=f"wkload{it}", bufs=3) as wkload, \
             tc.tile_pool(name=f"wkr{it}", bufs=1) as wkrp, \
             tc.tile_pool(name=f"refload{it}", bufs=4) as refload, \
             tc.tile_pool(name=f"ssb{it}", bufs=1) as ssbp, \
             tc.tile_pool(name=f"usb{it}", bufs=1) as usbp:
            Wk_r = []
            for cc in range(CC):
                w = wkload.tile([128, D], F32, tag="wkl")
                nc.sync.dma_start(out=w, in_=t_Wk.ap()[cc * 128:(cc + 1) * 128, :])
                wr = wkrp.tile([128, D], F32R, tag=f"wkr{cc}")
                nc.vector.tensor_copy(wr[:], w[:])
                Wk_r.append(wr)
            S_sb = [ssbp.tile([128, D], F32R, tag=f"S{cc}", name=f"S{it}_{cc}")
                    for cc in range(CC)]
            for p, cc1s in ((0, (0, 1, 2, 3)), (1, (4, 5))):
                with tc.tile_pool(name=f"sps0{it}_{p}", bufs=1, space="PSUM") as sp:
                    ps = {}
                    for cc1 in cc1s:
                        for jh in range(2):
                            ps[(cc1, jh)] = sp.tile(
                                [128, 384], F32, tag=f"ps{cc1}_{jh}",
                                name=f"ps{it}_{p}_{cc1}_{jh}")
                    for mch in range(2 * R):
                        rl = refload.tile([128, D], F32, tag="rl")
                        nc.sync.dma_start(
                            out=rl, in_=t_ref.ap()[mch * 128:(mch + 1) * 128, :])
                        rlr = refload.tile([128, D], F32R, tag="rlr")
                        if mch % 2 == 0:
                            nc.vector.tensor_copy(rlr[:], rl[:])
                        else:
                            nc.scalar.copy(rlr[:], rl[:])
                        for cc1 in cc1s:
                            for jh in range(2):
                                nc.tensor.matmul(
                                    ps[(cc1, jh)][:],
                                    rlr[:, cc1 * 128:(cc1 + 1) * 128],
                                    rlr[:, jh * 384:(jh + 1) * 384],
                                    start=(mch == 0), stop=(mch == 2 * R - 1))
                    for cc1 in cc1s:
                        for jh in range(2):
                            if (cc1 + jh) % 2 == 0:
                                nc.vector.tensor_copy(
                                    S_sb[cc1][:, jh * 384:(jh + 1) * 384], ps[(cc1, jh)][:])
                            else:
                                nc.scalar.copy(
                                    S_sb[cc1][:, jh * 384:(jh + 1) * 384], ps[(cc1, jh)][:])
            # U = S @ Wk (c1-part, j-free), zero-padded to 832 cols
            ups_ctx = ExitStack()
            ups = ups_ctx.enter_context(
                tc.tile_pool(name=f"ups{it}", bufs=3, space="PSUM"))
            gps = ups_ctx.enter_context(
                tc.tile_pool(name=f"gps{it}", bufs=2, space="PSUM"))
            U_sb = [usbp.tile([128, D + HD], F32R, tag=f"U{cc}", name=f"U{it}_{cc}")
                    for cc in range(CC)]
            for cc1 in range(CC):
                nc.scalar.activation(U_sb[cc1][:, D:D + HD], zsrc[:, 0:64],
                                     AF.Copy, scale=0.0)
                for jh in range(2):
                    pu = ups.tile([128, 384], F32, tag="pu")
                    for kc in range(CC):
                        nc.tensor.matmul(
                            pu[:], S_sb[kc][:, cc1 * 128:(cc1 + 1) * 128],
                            Wk_r[kc][:, jh * 384:(jh + 1) * 384],
                            start=(kc == 0), stop=(kc == CC - 1))
                    if jh == 0:
                        nc.vector.tensor_copy(U_sb[cc1][:, 0:384], pu[:])
                    else:
                        nc.scalar.copy(U_sb[cc1][:, 384:768], pu[:])
            # Gbar_h = Wk_h^T U_h: pair-block lhsT puts head h's Gram block
            # on rows par*64..; drain to the par-diagonal block of G_sb
            for h in range(H):
                pair, par = h // 2, h % 2
                pg = gps.tile([128, HD], F32, tag="pg", name=f"pg{it}_{h}")
                for kc in range(CC):
                    nc.tensor.matmul(
                        pg[:], Wk_r[kc][:, pair * 128: (pair + 1) * 128],
                        U_sb[kc][:, h * 64: h * 64 + 64],
                        start=(kc == 0), stop=(kc == CC - 1))
                nc.scalar.activation(
                    G_sb[h][par * 64:(par + 1) * 64, par * 64:(par + 1) * 64],
                    pg[par * 64:(par + 1) * 64, :], AF.Copy, scale=GS)
            ups_ctx.close()

        wqv_ctx = ExitStack()
        wqvres = wqv_ctx.enter_context(tc.tile_pool(name=f"wqvres{it}", bufs=1))
        wqv_stage = wqv_ctx.enter_context(tc.tile_pool(name=f"wqvstage{it}", bufs=3))
        Wqv_r = []
        for kc in range(CC):
            wl = wqv_stage.tile([128, 2 * D], F32, tag="wqvl", name=f"wqvl{it}_{kc}")
            nc.sync.dma_start(out=wl, in_=t_Wqv.ap()[kc * 128:(kc + 1) * 128, :])
            wr = wqvres.tile([128, 2 * D], F32R, tag=f"wqvr{kc}", name=f"wqvr{it}_{kc}")
            nc.vector.tensor_copy(wr[:], wl[:])
            Wqv_r.append(wr)
        for step in range(STEPS):
            # ---- A: qv^T = Wqv^T @ x^T ----
            with tc.tile_pool(name=f"qvps{it}_{step}", bufs=1, space="PSUM") as qvps, \
                 tc.tile_pool(name=f"vtps{it}_{step}", bufs=2, space="PSUM") as vtps, \
                 tc.tile_pool(name=f"vtmp{it}_{step}", bufs=2) as vtmp:
                for half in range(2):
                    pqv = [qvps.tile([128, N], F32, tag=f"pqv{j}",
                                     name=f"pqv{it}_{step}_{half}_{j}") for j in range(CC)]
                    for kc in range(CC):
                        for j in range(CC):
                            nc.tensor.matmul(
                                pqv[j][:],
                                Wqv_r[kc][:, half * D + j * 128: half * D + (j + 1) * 128],
                                xT[kc][:],
                                start=(kc == 0), stop=(kc == CC - 1))
                    for j in range(CC):
                        if half == 0:
                            nc.scalar.copy(qT[j][:], pqv[j][:])
                        else:
                            vt = vtmp.tile([128, N], F32, tag="vt")
                            nc.scalar.copy(vt[:], pqv[j][:])
                            for nch in range(2):
                                pt = vtps.tile([128, 128], F32, tag="vpt")
                                nc.tensor.transpose(pt[:], vt[:, nch * 128:(nch + 1) * 128],
                                                    ident[:])
                                nc.vector.tensor_copy(
                                    v_pad[nch][:, (2 * j) * 128 + 0:(2 * j) * 128 + 64],
                                    pt[:, 0:64])
                                nc.vector.tensor_copy(
                                    v_pad[nch][:, (2 * j + 1) * 128 + 64:(2 * j + 1) * 128 + 128],
                                    pt[:, 64:128])

            # ---- attention: q-nat transposes, w, P, z ----
            with tc.tile_pool(name=f"zps{it}_{step}", bufs=1, space="PSUM") as zps, \
                 tc.tile_pool(name=f"sps2{it}_{step}", bufs=1, space="PSUM") as sps2, \
                 tc.tile_pool(name=f"wps{it}_{step}", bufs=2, space="PSUM") as wps:
                zpsum = zps.tile([128, H * 128], F32, tag="z",
                                 name=f"zpsum{it}_{step}")  # 3 banks
                zbank_start = {}
                # q natural: one full-tile transpose per (pair, nch); the two
                # heads come out side by side in the free dim
                for pair in range(CC):
                    for nch in range(2):
                        pt = sps2.tile([128, 128], F32, tag="qnt")
                        nc.tensor.transpose(
                            pt[:], qT[pair][:, nch * 128:(nch + 1) * 128].bitcast(F32),
                            ident[:])
                        nc.vector.tensor_copy(
                            qn[2 * pair][:, nch * 128: nch * 128 + 64], pt[:, 0:64])
                        nc.vector.tensor_copy(
                            qn[2 * pair + 1][:, nch * 128 + 64: nch * 128 + 128],
                            pt[:, 64:128])
                for h in range(H):
                    pair, par = h // 2, h % 2
                    # qg^T = Gbar q^T (needs only A output; runs parallel to w)
                    pqg = wps.tile([128, N], F32, tag="qg", name=f"pqg{it}_{step}_{h}")
                    nc.tensor.matmul(pqg[:], G_sb[h][:], qT[pair][:],
                                     start=True, stop=True)
                    if par == 0:
                        nc.vector.tensor_copy(qg_sb[h][:], pqg[:])
                    else:
                        nc.scalar.copy(qg_sb[h][:], pqg[:])
                    # w = q^T v  (rows par*64.. via qn col-parity placement)
                    pw = wps.tile([128, HD], F32, tag="w", name=f"pw{it}_{step}_{h}")
                    for nch in range(2):
                        nc.tensor.matmul(
                            pw[:], qn[h][:, nch * 128:(nch + 1) * 128],
                            v_pad[nch][:, h * 128 + par * 64: h * 128 + (par + 1) * 64],
                            start=(nch == 0), stop=(nch == 1))
                    nc.scalar.copy(Pz[h][:, par * 64:(par + 1) * 64], pw[:])
                    # z^T[pair] += w^T-as-lhsT @ qg^T (par-packed output rows)
                    bank = pair // 2
                    is_start = (par == 0 and pair % 2 == 0)
                    mi = nc.tensor.matmul(
                        zpsum[:, pair * 256:(pair + 1) * 256],
                        Pz[h][:], qg_sb[h][:],
                        start=is_start,
                        stop=(par == 1 and pair % 2 == 1),
                        skip_group_check=True)
                    if is_start:
                        zbank_start[bank] = mi.ins
                    elif par == 0 and pair % 2 == 1:
                        add_dep_helper(mi.ins, zbank_start[bank], sync=False,
                                       reason="z region first-write after bank start")

                # ---- E: drain z + duplicate halves + strided regather ----
                for h in range(H):
                    par = h % 2
                    dst = m_sb[par * 64:(par + 1) * 64, h * N:(h + 1) * N]
                    src = zpsum[par * 64:(par + 1) * 64, (h // 2) * 256:(h // 2) * 256 + N]
                    if par == 0:
                        nc.vector.tensor_copy(dst, src)
                    else:
                        nc.scalar.copy(dst, src)
                ev = m_sb[0:64, :].rearrange("p (h n) -> p h n", n=N)[:, 0::2, :]
                ev_d = m_sb[64:128, :].rearrange("p (h n) -> p h n", n=N)[:, 0::2, :]
                od = m_sb[64:128, :].rearrange("p (h n) -> p h n", n=N)[:, 1::2, :]
                od_d = m_sb[0:64, :].rearrange("p (h n) -> p h n", n=N)[:, 1::2, :]
                nc.sync.dma_start(out=ev_d, in_=ev)
                nc.sync.dma_start(out=od_d, in_=od)
                for cc in range(CC):
                    nc.vector.tensor_copy(zT[cc][0:64, :], m_sb[0:64, 2 * cc::12])
                    nc.scalar.copy(zT[cc][64:128, :], m_sb[64:128, 2 * cc + 1::12])

            # ---- F: xp^T = Wproj^T @ z^T (+bproj);  G: LayerNorm over c ----
            with tc.tile_pool(name=f"fps{it}_{step}", bufs=2, space="PSUM") as fps, \
                 tc.tile_pool(name=f"sps{it}_{step}", bufs=2, space="PSUM") as sps, \
                 tc.tile_pool(name=f"ln{it}_{step}", bufs=1) as ln:
                for mc in range(CC):
                    pxp = fps.tile([128, N], F32, tag="pxp")
                    for kc in range(CC):
                        nc.tensor.matmul(
                            pxp[:], Wproj_r[kc][:, mc * 128:(mc + 1) * 128], zT[kc][:],
                            start=(kc == 0), stop=(kc == CC - 1))
                    nc.scalar.activation(
                        xp_sb[mc][:], pxp[:], AF.Identity, bias=bproj_sb[:, mc:mc + 1])
                for mc in range(CC):
                    nc.scalar.activation(sq_sb[mc][:], xp_sb[mc][:].bitcast(F32), AF.Square)
                psum_s = sps.tile([128, N], F32, tag="s", name=f"psum_s{it}_{step}")
                psum_q = sps.tile([128, N], F32, tag="q", name=f"psum_q{it}_{step}")
                for mc in range(CC):
                    nc.tensor.matmul(psum_s[:], ones128[:], xp_sb[mc][:],
                                     start=(mc == 0), stop=(mc == CC - 1))
                for mc in range(CC):
                    nc.tensor.matmul(psum_q[:], ones128[:], sq_sb[mc][:],
                                     start=(mc == 0), stop=(mc == CC - 1))
                mean_b = ln.tile([128, N], F32, tag="meanb")
                mean2_b = ln.tile([128, N], F32, tag="mean2b")
                var_b = ln.tile([128, N], F32, tag="varb")
                rsig_b = ln.tile([128, N], F32, tag="rsigb")
                nc.scalar.activation(mean_b[:], psum_s[:], AF.Copy, scale=1.0 / D)
                nc.vector.tensor_mul(mean2_b[:], mean_b[:], mean_b[:])
                nc.vector.scalar_tensor_tensor(
                    out=var_b[:], in0=psum_q[:], scalar=1.0 / D, in1=mean2_b[:],
                    op0=mybir.AluOpType.mult, op1=mybir.AluOpType.subtract)
                nc.scalar.activation(var_b[:], var_b[:], AF.Sqrt, bias=eps_sb[:])
                nc.vector.reciprocal(rsig_b[:], var_b[:])
                tmp = ln.tile([128, N], F32, tag="lntmp")
                for mc in range(CC):
                    nc.vector.tensor_sub(tmp[:], xp_sb[mc][:].bitcast(F32), mean_b[:])
                    nc.vector.scalar_tensor_tensor(
                        out=tmp[:], in0=tmp[:], scalar=gamma_sb[:, mc:mc + 1],
                        in1=rsig_b[:],
                        op0=mybir.AluOpType.mult, op1=mybir.AluOpType.mult)
                    nc.vector.tensor_scalar_add(
                        out=xT[mc][:], in0=tmp[:], scalar1=beta_sb[:, mc:mc + 1])

        wqv_ctx.close()

        # ---- epilogue: transpose x^T -> x, store ----
        with tc.tile_pool(name=f"eps{it}", bufs=3, space="PSUM") as eps_pool, \
             tc.tile_pool(name=f"osb{it}", bufs=1) as osb:
            out_nat = [osb.tile([128, D], F32, tag=f"on{it}_{nch}",
                                name=f"on{it}_{nch}") for nch in range(2)]
            for cc in range(CC):
                for nch in range(2):
                    pt = eps_pool.tile([128, 128], F32, tag="ept")
                    nc.tensor.transpose(
                        pt[:], xT[cc][:, nch * 128:(nch + 1) * 128].bitcast(F32), ident[:])
                    nc.vector.tensor_copy(out_nat[nch][:, cc * 128:(cc + 1) * 128], pt[:])
            for nch in range(2):
                nc.sync.dma_start(out=t_out.ap()[nch * 128:(nch + 1) * 128, :],
                                  in_=out_nat[nch][:])

    if iters == 1:
        one_pass(0)
    else:
        with tc.For_i(0, iters, 1):
            one_pass(0)


def build(iters=1):
    nc = bacc.Bacc("TRN2", target_bir_lowering=False, debug=False, num_devices=NB)
    t_x = nc.declare_dram_parameter("x", [N, D], F32, isOutput=False)
    t_ref = nc.declare_dram_parameter("ref", [R * N, D], F32, isOutput=False)
    t_Wqv = nc.declare_dram_parameter("Wqv", [D, 2 * D], F32, isOutput=False)
    t_Wk = nc.declare_dram_parameter("Wk", [D, D], F32, isOutput=False)
    t_Wproj = nc.declare_dram_parameter("Wproj", [D, D], F32, isOutput=False)
    t_bproj = nc.declare_dram_parameter("bproj", [D], F32, isOutput=False)
    t_gamma = nc.declare_dram_parameter("gamma", [D], F32, isOutput=False)
    t_beta = nc.declare_dram_parameter("beta", [D], F32, isOutput=False)
    t_out = nc.declare_dram_parameter("out", [N, D], F32, isOutput=True)
    with tile.TileContext(nc) as tc:
        with ExitStack() as ctx:
            _emit(nc, tc, ctx, t_x, t_ref, t_Wqv, t_Wk, t_Wproj, t_bproj,
                  t_gamma, t_beta, t_out, iters=iters)
    nc.compile()
    return nc


_CACHE = {}
last_results = None


def kernel(x, ref, Wqv, Wk, Wproj, bproj, gamma, beta):
    global last_results
    if "nc" not in _CACHE:
        _CACHE["nc"] = build()
    nc = _CACHE["nc"]

    def f(a):
        return np.ascontiguousarray(np.asarray(a), dtype=np.float32)

    x = f(x)
    common = dict(ref=f(ref).reshape(R * N, D), Wqv=f(Wqv), Wk=f(Wk),
                  Wproj=f(Wproj), bproj=f(bproj), gamma=f(gamma), beta=f(beta))
    in_maps = [dict(x=x[b], **common) for b in range(NB)]
    res = run_bass_kernel_spmd(nc, in_maps, list(range(NB)))
    last_results = res
    return np.stack([res.results[b]["out"] for b in range(NB)]).astype(np.float32)



# revision 11
# speedup vs baseline: 1.4401x; 1.4401x over previous
"""Trainium2 Bass kernel for nn_Diffuser (sparse_attention), v3.

Algebra: the reference attention has NO softmax, so per head
    mean_r y_rh = q_h @ Gbar_h @ (q_h^T v_h),
    Gbar_h = s^2/R * sum_r K_rh^T K_rh,  K = ref @ Wk   (64x64 per head).

v3 design:
  - bf16 matmul inputs (f32 PSUM accumulation); host pre-casts and
    pre-transposes (layout-only prep).  All matmuls 1 cycle/row + FWL.
  - Replicated Gbar precompute via triangular S = ref^T ref: only the 21
    upper tiles (one 8-bank PSUM pass over 20 streamed chunks), mirrors
    by PE transpose, U = S@Wk per pair, pair-diagonal G extraction.
    (A 196KB AllReduce alone measures ~90-110us on this 8-core setup —
    replication is strictly faster.)
  - LayerNorm folded into the next step's QV matmul: bias via K=1
    ones-row matmuls into the F PSUM, bn_stats on PSUM, drain scaled by
    rsig, and mean/beta corrections as two outer-product rows appended
    to the QV contraction (Wqv is gamma-scaled in-place on device after
    step 0).  Only the final step materializes the normalized output.
"""

import numpy as np
import ml_dtypes
from contextlib import ExitStack

import concourse.bass as bass
import concourse.tile as tile
from concourse import bacc, mybir
from concourse.bass_utils import run_bass_kernel_spmd
from concourse.masks import make_identity

F32 = mybir.dt.float32
BF16 = mybir.dt.bfloat16
AF = mybir.ActivationFunctionType

D = 768
H = 12
HD = 64
R = 10
N = 256
STEPS = 3
NB = 8
CC = D // 128  # 6
SCALE = HD ** -0.5
EPS = 1e-5
GS = SCALE * SCALE / R  # folded into Gbar
NCH = (R * N) // 128    # 20 ref chunks


def _emit(nc, tc, ctx, t_xT, t_ref, t_Wqv, t_Wk, t_Wproj, t_gammab, t_betab,
          t_gcol, t_gbq, t_bprow, t_out):
    const = ctx.enter_context(tc.tile_pool(name="const", bufs=1))
    persist = ctx.enter_context(tc.tile_pool(name="persist", bufs=1))

    eps_sb = const.tile([128, 1], F32)
    nc.vector.memset(eps_sb, EPS)
    ident_bf = const.tile([128, 128], BF16)
    make_identity(nc, ident_bf)

    # ---- input DMAs in order of first use (ref chunks feed the Gram) ----
    refp = ctx.enter_context(tc.tile_pool(name="refp", bufs=1))
    ref_ch = [refp.tile([128, D], BF16, tag=f"rc{m}", name=f"rc{m}")
              for m in range(NCH)]
    for m in range(NCH):
        nc.sync.dma_start(out=ref_ch[m], in_=t_ref.ap()[m * 128:(m + 1) * 128, :])
    wk_sb = [persist.tile([128, D], BF16, tag=f"wk{k}", name=f"wk{k}")
             for k in range(CC)]
    xT_sb = [persist.tile([128, N], BF16, tag=f"xT{k}", name=f"xT{k}")
             for k in range(CC)]
    Wqv_sb = [persist.tile([128, 2 * D], BF16, tag=f"wqv{k}", name=f"wqv{k}")
              for k in range(CC)]
    Wproj_sb = [persist.tile([128, D], BF16, tag=f"wp{k}", name=f"wp{k}")
                for k in range(CC)]
    for k in range(CC):
        nc.sync.dma_start(out=wk_sb[k], in_=t_Wk.ap()[k * 128:(k + 1) * 128, :])
    for k in range(CC):
        nc.sync.dma_start(out=xT_sb[k], in_=t_xT.ap()[k * 128:(k + 1) * 128, :])
        nc.sync.dma_start(out=Wqv_sb[k], in_=t_Wqv.ap()[k * 128:(k + 1) * 128, :])
    for k in range(CC):
        nc.sync.dma_start(out=Wproj_sb[k], in_=t_Wproj.ap()[k * 128:(k + 1) * 128, :])
    gamma_sb = const.tile([128, D], F32)
    beta_sb = const.tile([128, D], F32)
    gcol_sb = const.tile([128, CC], F32)
    gbq_sb = const.tile([2, 2 * D], BF16)
    bprow_sb = const.tile([1, D], BF16)
    nc.sync.dma_start(out=gcol_sb, in_=t_gcol.ap())
    nc.sync.dma_start(out=gbq_sb, in_=t_gbq.ap())
    nc.sync.dma_start(out=bprow_sb, in_=t_bprow.ap())
    nc.sync.dma_start(out=gamma_sb, in_=t_gammab.ap())
    nc.sync.dma_start(out=beta_sb, in_=t_betab.ap())

    # ---- persistent state ----
    q_sb = [persist.tile([128, D], BF16, tag=f"q{n}", name=f"q{n}") for n in range(2)]
    v_sb = [persist.tile([128, D], BF16, tag=f"v{n}", name=f"v{n}") for n in range(2)]
    qT_sb = [persist.tile([128, N], BF16, tag=f"qT{k}", name=f"qT{k}")
             for k in range(CC)]
    wb_sb = persist.tile([128, H * 32], BF16, tag="wb", name="wb")   # [128, 384]
    Pb_sb = persist.tile([128, H * 32], BF16, tag="Pb", name="Pb")
    G_bf = persist.tile([128, H * 32], BF16, tag="Gbf", name="Gbf")
    m_sb = persist.tile([128, H * N], BF16, tag="m_sb", name="m_sb")
    zT_sb = [persist.tile([128, N], BF16, tag=f"zT{k}", name=f"zT{k}")
             for k in range(CC)]
    tmp_n = [persist.tile([128, D], F32, tag=f"tmpn{n}", name=f"tmpn{n}")
             for n in range(2)]
    xnat_b = [persist.tile([128, D], BF16, tag=f"xn{n}", name=f"xn{n}")
              for n in range(2)]
    out_sb = [persist.tile([128, D], F32, tag=f"os{n}", name=f"os{n}")
              for n in range(2)]
    stats = [persist.tile([128, 3, 6], F32, tag=f"st{n}", name=f"st{n}")
             for n in range(2)]
    mv = [persist.tile([128, 2], F32, tag=f"mv{n}", name=f"mv{n}") for n in range(2)]
    rsig = [persist.tile([128, 1], F32, tag=f"rs{n}", name=f"rs{n}")
            for n in range(2)]
    rm_bf = [persist.tile([128, 1], BF16, tag=f"rm{n}", name=f"rm{n}")
             for n in range(2)]
    aug_sb = persist.tile([2, N], BF16, tag="aug", name="aug")
    rmtmp = persist.tile([1, N], BF16, tag="rmtmp", name="rmtmp")
    nc.vector.memset(aug_sb[0:1, :], 1.0)

    # ---- precompute (replicated): Gbar via triangular S = ref^T ref ----
    ssb = ctx.enter_context(tc.tile_pool(name="ssb", bufs=1))
    mirp = ctx.enter_context(tc.tile_pool(name="mirp", bufs=1))
    usb = ctx.enter_context(tc.tile_pool(name="usb", bufs=1))
    with tc.tile_pool(name="sps", bufs=1, space="PSUM") as sps:
        S_ps = [sps.tile([128, D - 128 * c1], F32, tag=f"s{c1}", name=f"s{c1}")
                for c1 in range(CC)]
        for m in range(NCH):
            for c1 in range(CC):
                wleft = D - 128 * c1
                for off in range(0, wleft, 512):
                    w = min(512, wleft - off)
                    nc.tensor.matmul(
                        S_ps[c1][:, off:off + w],
                        ref_ch[m][:, c1 * 128:(c1 + 1) * 128],
                        ref_ch[m][:, c1 * 128 + off:c1 * 128 + off + w],
                        start=(m == 0), stop=(m == NCH - 1))
        S_sb = [ssb.tile([128, D - 128 * c1], BF16, tag=f"sb{c1}", name=f"sb{c1}")
                for c1 in range(CC)]
        for c1 in range(CC):
            if c1 % 2 == 0:
                nc.vector.tensor_copy(S_sb[c1][:], S_ps[c1][:])
            else:
                nc.scalar.copy(S_sb[c1][:], S_ps[c1][:])
    with tc.tile_pool(name="ptp", bufs=2, space="PSUM") as ptp, \
         tc.tile_pool(name="ups", bufs=3, space="PSUM") as ups:
        mir_sb = [mirp.tile([128, kc * 128], BF16, tag=f"mir{kc}", name=f"mir{kc}")
                  for kc in range(1, CC)]
        for kc in range(1, CC):
            for c1 in range(kc):
                pt = ptp.tile([128, 128], BF16, tag="pt", name="pt")
                nc.tensor.transpose(
                    pt[:], S_sb[c1][:, (kc - c1) * 128:(kc - c1 + 1) * 128],
                    ident_bf[:])
                if (kc + c1) % 2 == 0:
                    nc.vector.tensor_copy(
                        mir_sb[kc - 1][:, c1 * 128:(c1 + 1) * 128], pt[:])
                else:
                    nc.scalar.copy(
                        mir_sb[kc - 1][:, c1 * 128:(c1 + 1) * 128], pt[:])
        U_sb = [usb.tile([128, 128], BF16, tag=f"u{kc}", name=f"u{kc}")
                for kc in range(CC)]
        # U columns are needed pair-by-pair: U[:, p*128:+128], then the
        # pair-G product whose diagonal quadrants are G_{2p}, G_{2p+1}.
        with tc.tile_pool(name="gps2", bufs=2, space="PSUM") as gps2:
            for p in range(CC):
                for kc in range(CC):
                    up = ups.tile([128, 128], F32, tag="up", name="up")
                    for k2 in range(CC):
                        if k2 <= kc:
                            lhsT = S_sb[k2][:, (kc - k2) * 128:(kc - k2 + 1) * 128]
                        else:
                            lhsT = mir_sb[k2 - 1][:, kc * 128:(kc + 1) * 128]
                        nc.tensor.matmul(up[:], lhsT,
                                         wk_sb[k2][:, p * 128:(p + 1) * 128],
                                         start=(k2 == 0), stop=(k2 == CC - 1))
                    if kc % 2 == 0:
                        nc.vector.tensor_copy(U_sb[kc][:], up[:])
                    else:
                        nc.scalar.copy(U_sb[kc][:], up[:])
                gp = gps2.tile([128, 128], F32, tag="gp", name="gp")
                for kc in range(CC):
                    nc.tensor.matmul(gp[:], wk_sb[kc][:, p * 128:(p + 1) * 128],
                                     U_sb[kc][:],
                                     start=(kc == 0), stop=(kc == CC - 1))
                for par in range(2):
                    sl = slice(par * 64, (par + 1) * 64)
                    if par == 0:
                        nc.vector.tensor_scalar_mul(
                            out=G_bf[sl, p * 64:(p + 1) * 64],
                            in0=gp[sl, sl], scalar1=GS)
                    else:
                        nc.scalar.activation(
                            G_bf[sl, p * 64:(p + 1) * 64], gp[sl, sl],
                            AF.Copy, scale=GS)

    # ---- diffusion steps ----
    for s in range(STEPS):
        # A: qv natural = x @ Wqv via xT-stationary matmuls.  Steps 1-2:
        # xT holds rsig-scaled pre-LN x; Wqv is gamma-scaled; two extra
        # outer-product rows add the -rm*gqv and bqv LayerNorm terms.
        with tc.tile_pool(name=f"qvps{s}", bufs=1, space="PSUM") as qvps:
            qv_ps = [qvps.tile([128, 2 * D], F32, tag=f"qv{n}", name=f"qv{n}")
                     for n in range(2)]
            for n in range(2):
                for kc in range(CC):
                    for m in range(3):
                        nc.tensor.matmul(
                            qv_ps[n][:, m * 512:(m + 1) * 512],
                            xT_sb[kc][:, n * 128:(n + 1) * 128],
                            Wqv_sb[kc][:, m * 512:(m + 1) * 512],
                            start=(kc == 0),
                            stop=(kc == CC - 1 and s == 0))
                if s > 0:
                    for m in range(3):
                        nc.tensor.matmul(
                            qv_ps[n][:, m * 512:(m + 1) * 512],
                            aug_sb[:, n * 128:(n + 1) * 128],
                            gbq_sb[:, m * 512:(m + 1) * 512],
                            start=False, stop=True)
            for n in range(2):
                nc.vector.tensor_copy(q_sb[n][:], qv_ps[n][:, 0:D])
                nc.scalar.copy(v_sb[n][:], qv_ps[n][:, D:2 * D])
        if s == 0:
            # fold gamma into Wqv for the remaining steps (in place)
            for kc in range(CC):
                nc.vector.tensor_scalar_mul(
                    out=Wqv_sb[kc][:], in0=Wqv_sb[kc][:],
                    scalar1=gcol_sb[:, kc:kc + 1])

        # q^T via PE transposes
        with tc.tile_pool(name=f"qtp{s}", bufs=3, space="PSUM") as qtp:
            for cc in range(CC):
                for n in range(2):
                    pt = qtp.tile([128, 128], BF16, tag="pt", name="pt")
                    nc.tensor.transpose(pt[:], q_sb[n][:, cc * 128:(cc + 1) * 128],
                                        ident_bf[:])
                    if (cc + n) % 2 == 0:
                        nc.vector.tensor_copy(qT_sb[cc][:, n * 128:(n + 1) * 128],
                                              pt[:])
                    else:
                        nc.scalar.copy(qT_sb[cc][:, n * 128:(n + 1) * 128], pt[:])

        # attention: w = q^T v, P = G w, z^T = P^T q^T
        with tc.tile_pool(name=f"wps{s}", bufs=1, space="PSUM") as wps, \
             tc.tile_pool(name=f"pps{s}", bufs=1, space="PSUM") as pps, \
             tc.tile_pool(name=f"zps{s}", bufs=1, space="PSUM") as zps:
            W_ps = wps.tile([128, H * 32], F32, tag="w", name="w")
            for h in range(H):
                par, pr = h % 2, h // 2
                for n in range(2):
                    nc.tensor.matmul(
                        W_ps[par * 64:(par + 1) * 64, pr * 64:(pr + 1) * 64],
                        q_sb[n][:, h * 64:(h + 1) * 64],
                        v_sb[n][:, h * 64:(h + 1) * 64],
                        start=(n == 0), stop=(n == 1))
            nc.vector.tensor_copy(wb_sb[:], W_ps[:])
            P_ps = pps.tile([128, H * 32], F32, tag="p", name="p")
            for h in range(H):
                par, pr = h % 2, h // 2
                sl = slice(par * 64, (par + 1) * 64)
                cl = slice(pr * 64, (pr + 1) * 64)
                nc.tensor.matmul(P_ps[sl, cl], G_bf[sl, cl], wb_sb[sl, cl],
                                 start=True, stop=True)
            nc.scalar.copy(Pb_sb[:], P_ps[:])
            z_ps = zps.tile([128, H * 128], F32, tag="z", name="z")  # 3 banks
            for h in range(H):
                par, pr = h % 2, h // 2
                sl = slice(par * 64, (par + 1) * 64)
                nc.tensor.matmul(
                    z_ps[sl, pr * 256:(pr + 1) * 256],
                    Pb_sb[sl, pr * 64:(pr + 1) * 64],
                    qT_sb[pr][sl, :],
                    start=True, stop=True)

            # E: drain z + duplicate parity halves + strided regather
            for h in range(H):
                par, pr = h % 2, h // 2
                sl = slice(par * 64, (par + 1) * 64)
                dst = m_sb[sl, h * N:(h + 1) * N]
                src = z_ps[sl, pr * 256:pr * 256 + N]
                if par == 0:
                    nc.vector.tensor_copy(dst, src)
                else:
                    nc.scalar.copy(dst, src)
        ev = m_sb[0:64, :].rearrange("p (h n) -> p h n", n=N)[:, 0::2, :]
        ev_d = m_sb[64:128, :].rearrange("p (h n) -> p h n", n=N)[:, 0::2, :]
        od = m_sb[64:128, :].rearrange("p (h n) -> p h n", n=N)[:, 1::2, :]
        od_d = m_sb[0:64, :].rearrange("p (h n) -> p h n", n=N)[:, 1::2, :]
        nc.sync.dma_start(out=ev_d, in_=ev)
        nc.sync.dma_start(out=od_d, in_=od)
        for cc in range(CC):
            nc.vector.tensor_copy(zT_sb[cc][0:64, :], m_sb[0:64, 2 * cc::12])
            nc.scalar.copy(zT_sb[cc][64:128, :], m_sb[64:128, 2 * cc + 1::12])

        # F: xb = z @ Wproj + bproj in PSUM (ones-row K=1 matmul adds the
        # bias); LN stats straight off PSUM.
        with tc.tile_pool(name=f"fps{s}", bufs=1, space="PSUM") as fps:
            xp_ps = [fps.tile([128, D], F32, tag=f"xp{n}", name=f"xp{n}")
                     for n in range(2)]
            for n in range(2):
                for kc in range(CC):
                    for off, w in ((0, 512), (512, 256)):
                        nc.tensor.matmul(xp_ps[n][:, off:off + w],
                                         zT_sb[kc][:, n * 128:(n + 1) * 128],
                                         Wproj_sb[kc][:, off:off + w],
                                         start=(kc == 0), stop=False)
                for off, w in ((0, 512), (512, 256)):
                    nc.tensor.matmul(xp_ps[n][:, off:off + w],
                                     aug_sb[0:1, n * 128:(n + 1) * 128],
                                     bprow_sb[:, off:off + w],
                                     start=False, stop=True)
            for n in range(2):
                xv = xp_ps[n][:].rearrange("p (a b) -> p a b", b=256)
                for g in range(3):
                    nc.vector.bn_stats(out=stats[n][:, g, :], in_=xv[:, g, :])
                nc.vector.bn_aggr(out=mv[n][:], in_=stats[n][:])
                nc.scalar.activation(rsig[n][:], mv[n][:, 1:2], AF.Sqrt,
                                     bias=eps_sb[:])
                nc.vector.reciprocal(rsig[n][:], rsig[n][:])
                if s < STEPS - 1:
                    # drain pre-LN x scaled by rsig; mean/beta terms are
                    # folded into the next A matmul via aug rows
                    nc.vector.tensor_scalar_mul(
                        out=xnat_b[n][:], in0=xp_ps[n][:],
                        scalar1=rsig[n][:, 0:1])
                    nc.vector.tensor_mul(rm_bf[n][:], mv[n][:, 0:1], rsig[n][:])
                else:
                    nc.vector.scalar_tensor_tensor(
                        out=tmp_n[n][:], in0=xp_ps[n][:], scalar=mv[n][:, 0:1],
                        in1=gamma_sb[:],
                        op0=mybir.AluOpType.subtract, op1=mybir.AluOpType.mult)
                    nc.vector.scalar_tensor_tensor(
                        out=out_sb[n][:], in0=tmp_n[n][:], scalar=rsig[n][:, 0:1],
                        in1=beta_sb[:],
                        op0=mybir.AluOpType.mult, op1=mybir.AluOpType.add)
        if s < STEPS - 1:
            with tc.tile_pool(name=f"xtp{s}", bufs=3, space="PSUM") as xtp:
                for n in range(2):
                    ptr = xtp.tile([1, 128], BF16, tag="ptr", name="ptr")
                    nc.tensor.transpose(ptr[:], rm_bf[n][:], ident_bf[:])
                    nc.scalar.copy(rmtmp[0:1, n * 128:(n + 1) * 128], ptr[:])
                nc.sync.dma_start(out=aug_sb[1:2, :], in_=rmtmp[0:1, :])
                for cc in range(CC):
                    for n in range(2):
                        pt = xtp.tile([128, 128], BF16, tag="pt", name="pt")
                        nc.tensor.transpose(
                            pt[:], xnat_b[n][:, cc * 128:(cc + 1) * 128],
                            ident_bf[:])
                        if (cc + n) % 2 == 0:
                            nc.vector.tensor_copy(
                                xT_sb[cc][:, n * 128:(n + 1) * 128], pt[:])
                        else:
                            nc.scalar.copy(
                                xT_sb[cc][:, n * 128:(n + 1) * 128], pt[:])
        else:
            for n in range(2):
                nc.sync.dma_start(out=t_out.ap()[n * 128:(n + 1) * 128, :],
                                  in_=out_sb[n][:])


def build():
    nc = bacc.Bacc("TRN2", target_bir_lowering=False, debug=False, num_devices=NB)
    t_xT = nc.declare_dram_parameter("xT", [D, N], BF16, isOutput=False)
    t_ref = nc.declare_dram_parameter("ref", [R * N, D], BF16, isOutput=False)
    t_Wqv = nc.declare_dram_parameter("Wqv", [D, 2 * D], BF16, isOutput=False)
    t_Wk = nc.declare_dram_parameter("Wk", [D, D], BF16, isOutput=False)
    t_Wproj = nc.declare_dram_parameter("Wproj", [D, D], BF16, isOutput=False)
    t_gammab = nc.declare_dram_parameter("gammab", [128, D], F32, isOutput=False)
    t_betab = nc.declare_dram_parameter("betab", [128, D], F32, isOutput=False)
    t_gcol = nc.declare_dram_parameter("gcol", [128, CC], F32, isOutput=False)
    t_gbq = nc.declare_dram_parameter("gbq", [2, 2 * D], BF16, isOutput=False)
    t_bprow = nc.declare_dram_parameter("bprow", [1, D], BF16, isOutput=False)
    t_out = nc.declare_dram_parameter("out", [N, D], F32, isOutput=True)
    with tile.TileContext(nc) as tc:
        with ExitStack() as ctx:
            _emit(nc, tc, ctx, t_xT, t_ref, t_Wqv, t_Wk, t_Wproj, t_gammab,
                  t_betab, t_gcol, t_gbq, t_bprow, t_out)
    nc.compile()
    return nc


_CACHE = {}
last_results = None


def kernel(x, ref, Wqv, Wk, Wproj, bproj, gamma, beta):
    global last_results
    if "nc" not in _CACHE:
        _CACHE["nc"] = build()
    nc = _CACHE["nc"]

    bf = ml_dtypes.bfloat16

    def f32(a):
        return np.ascontiguousarray(np.asarray(a), dtype=np.float32)

    x = f32(x)
    ref_flat = f32(ref).reshape(R * N, D)
    Wqv_f = f32(Wqv)
    gamma_f = f32(gamma)
    beta_f = f32(beta)
    bproj_f = f32(bproj)
    gbq = np.stack([beta_f @ Wqv_f, -(gamma_f @ Wqv_f)]).astype(bf)
    common = dict(
        ref=np.ascontiguousarray(ref_flat.astype(bf)),
        Wqv=np.ascontiguousarray(Wqv_f.astype(bf)),
        Wk=np.ascontiguousarray(f32(Wk).astype(bf)),
        Wproj=np.ascontiguousarray(f32(Wproj).astype(bf)),
        gammab=np.ascontiguousarray(np.broadcast_to(gamma_f, (128, D))),
        betab=np.ascontiguousarray(np.broadcast_to(beta_f, (128, D))),
        gcol=np.ascontiguousarray(gamma_f.reshape(CC, 128).T),
        gbq=np.ascontiguousarray(gbq),
        bprow=np.ascontiguousarray(bproj_f.astype(bf).reshape(1, D)),
    )
    in_maps = []
    for b in range(NB):
        xT = np.ascontiguousarray(x[b].T.astype(bf))
        in_maps.append(dict(xT=xT, **common))
    res = run_bass_kernel_spmd(nc, in_maps, list(range(NB)))
    last_results = res
    return np.stack([res.results[b]["out"] for b in range(NB)]).astype(np.float32)


# revision 14
# speedup vs baseline: 1.6925x; 1.1752x over previous
"""Trainium2 Bass kernel for nn_Diffuser (sparse_attention), v3.

Algebra: the reference attention has NO softmax, so per head
    mean_r y_rh = q_h @ Gbar_h @ (q_h^T v_h),
    Gbar_h = s^2/R * sum_r K_rh^T K_rh,  K = ref @ Wk   (64x64 per head).

v3 design:
  - bf16 matmul inputs (f32 PSUM accumulation); host pre-casts and
    pre-transposes (layout-only prep).  All matmuls 1 cycle/row + FWL.
  - Replicated Gbar precompute via triangular S = ref^T ref: only the 21
    upper tiles (one 8-bank PSUM pass over 20 streamed chunks), mirrors
    by PE transpose, U = S@Wk per pair, pair-diagonal G extraction.
    (A 196KB AllReduce alone measures ~90-110us on this 8-core setup —
    replication is strictly faster.)
  - LayerNorm folded into the next step's QV matmul: bias via K=1
    ones-row matmuls into the F PSUM, bn_stats on PSUM, drain scaled by
    rsig, and mean/beta corrections as two outer-product rows appended
    to the QV contraction (Wqv is gamma-scaled in-place on device after
    step 0).  Only the final step materializes the normalized output.
"""

import numpy as np
import ml_dtypes
from contextlib import ExitStack

import concourse.bass as bass
import concourse.tile as tile
from concourse import bacc, mybir
from concourse.bass_utils import run_bass_kernel_spmd
from concourse.masks import make_identity

F32 = mybir.dt.float32
BF16 = mybir.dt.bfloat16
AF = mybir.ActivationFunctionType

D = 768
H = 12
HD = 64
R = 10
N = 256
STEPS = 3
NB = 8
CC = D // 128  # 6
SCALE = HD ** -0.5
EPS = 1e-5
GS = SCALE * SCALE / R  # folded into Gbar
NCH = (R * N) // 128    # 20 ref chunks


def _emit(nc, tc, ctx, t_xT, t_ref, t_Wqv, t_Wk, t_Wproj, t_gammab, t_betab,
          t_gcol, t_gbq, t_bprow, t_out):
    const = ctx.enter_context(tc.tile_pool(name="const", bufs=1))
    persist = ctx.enter_context(tc.tile_pool(name="persist", bufs=1))

    eps_sb = const.tile([128, 1], F32)
    nc.vector.memset(eps_sb, EPS)
    ident_bf = const.tile([128, 128], BF16)
    make_identity(nc, ident_bf)

    # ---- input DMAs in order of first use (ref chunks feed the Gram) ----
    refp = ctx.enter_context(tc.tile_pool(name="refp", bufs=1))
    ref_ch = [refp.tile([128, D], BF16, tag=f"rc{m}", name=f"rc{m}")
              for m in range(NCH)]
    for m in range(NCH):
        nc.sync.dma_start(out=ref_ch[m], in_=t_ref.ap()[m * 128:(m + 1) * 128, :])
    wk_sb = [persist.tile([128, D], BF16, tag=f"wk{k}", name=f"wk{k}")
             for k in range(CC)]
    xT_sb = [persist.tile([128, N], BF16, tag=f"xT{k}", name=f"xT{k}")
             for k in range(CC)]
    Wqv_sb = [persist.tile([128, 2 * D], BF16, tag=f"wqv{k}", name=f"wqv{k}")
              for k in range(CC)]
    Wproj_sb = [persist.tile([128, D], BF16, tag=f"wp{k}", name=f"wp{k}")
                for k in range(CC)]
    for k in range(CC):
        nc.sync.dma_start(out=wk_sb[k], in_=t_Wk.ap()[k * 128:(k + 1) * 128, :])
    for k in range(CC):
        nc.sync.dma_start(out=xT_sb[k], in_=t_xT.ap()[k * 128:(k + 1) * 128, :])
        nc.sync.dma_start(out=Wqv_sb[k], in_=t_Wqv.ap()[k * 128:(k + 1) * 128, :])
    for k in range(CC):
        nc.sync.dma_start(out=Wproj_sb[k], in_=t_Wproj.ap()[k * 128:(k + 1) * 128, :])
    gamma_sb = const.tile([128, D], F32)
    beta_sb = const.tile([128, D], F32)
    gcol_sb = const.tile([128, CC], F32)
    gbq_sb = const.tile([2, 2 * D], BF16)
    bprow_sb = const.tile([1, D], BF16)
    nc.sync.dma_start(out=gcol_sb, in_=t_gcol.ap())
    nc.sync.dma_start(out=gbq_sb, in_=t_gbq.ap())
    nc.sync.dma_start(out=bprow_sb, in_=t_bprow.ap())
    nc.sync.dma_start(out=gamma_sb, in_=t_gammab.ap())
    nc.sync.dma_start(out=beta_sb, in_=t_betab.ap())

    # ---- persistent state ----
    q_sb = [persist.tile([128, D], BF16, tag=f"q{n}", name=f"q{n}") for n in range(2)]
    v_sb = [persist.tile([128, D], BF16, tag=f"v{n}", name=f"v{n}") for n in range(2)]
    qT_sb = [persist.tile([128, N], BF16, tag=f"qT{k}", name=f"qT{k}")
             for k in range(CC)]
    wb_sb = persist.tile([128, H * 32], BF16, tag="wb", name="wb")   # [128, 384]
    Pb_sb = persist.tile([128, H * 32], BF16, tag="Pb", name="Pb")
    G_bf = persist.tile([128, H * 32], BF16, tag="Gbf", name="Gbf")
    m_sb = persist.tile([128, H * N], BF16, tag="m_sb", name="m_sb")
    zT_sb = [persist.tile([128, N], BF16, tag=f"zT{k}", name=f"zT{k}")
             for k in range(CC)]
    tmp_n = [persist.tile([128, D], F32, tag=f"tmpn{n}", name=f"tmpn{n}")
             for n in range(2)]
    xnat_b = [persist.tile([128, D], BF16, tag=f"xn{n}", name=f"xn{n}")
              for n in range(2)]
    out_sb = [persist.tile([128, D], F32, tag=f"os{n}", name=f"os{n}")
              for n in range(2)]
    stats = [persist.tile([128, 3, 6], F32, tag=f"st{n}", name=f"st{n}")
             for n in range(2)]
    mv = [persist.tile([128, 2], F32, tag=f"mv{n}", name=f"mv{n}") for n in range(2)]
    rsig = [persist.tile([128, 1], F32, tag=f"rs{n}", name=f"rs{n}")
            for n in range(2)]
    rm_bf = [persist.tile([128, 1], BF16, tag=f"rm{n}", name=f"rm{n}")
             for n in range(2)]
    aug_sb = persist.tile([2, N], BF16, tag="aug", name="aug")
    rmtmp = persist.tile([1, N], BF16, tag="rmtmp", name="rmtmp")
    nc.vector.memset(aug_sb[0:1, :], 1.0)

    # ---- precompute (replicated): Gbar via triangular S = ref^T ref ----
    ssb = ctx.enter_context(tc.tile_pool(name="ssb", bufs=1))
    mirp = ctx.enter_context(tc.tile_pool(name="mirp", bufs=1))
    usb = ctx.enter_context(tc.tile_pool(name="usb", bufs=1))
    with tc.tile_pool(name="sps", bufs=1, space="PSUM") as sps:
        S_ps = [sps.tile([128, D - 128 * c1], F32, tag=f"s{c1}", name=f"s{c1}")
                for c1 in range(CC)]
        for m in range(NCH):
            for c1 in range(CC):
                wleft = D - 128 * c1
                for off in range(0, wleft, 512):
                    w = min(512, wleft - off)
                    nc.tensor.matmul(
                        S_ps[c1][:, off:off + w],
                        ref_ch[m][:, c1 * 128:(c1 + 1) * 128],
                        ref_ch[m][:, c1 * 128 + off:c1 * 128 + off + w],
                        start=(m == 0), stop=(m == NCH - 1))
        S_sb = [ssb.tile([128, D - 128 * c1], BF16, tag=f"sb{c1}", name=f"sb{c1}")
                for c1 in range(CC)]
        for c1 in range(CC):
            if c1 % 2 == 0:
                nc.vector.tensor_copy(S_sb[c1][:], S_ps[c1][:])
            else:
                nc.scalar.copy(S_sb[c1][:], S_ps[c1][:])
    with tc.tile_pool(name="ptp", bufs=2, space="PSUM") as ptp, \
         tc.tile_pool(name="ups", bufs=2, space="PSUM") as ups:
        mir_sb = [mirp.tile([128, kc * 128], BF16, tag=f"mir{kc}", name=f"mir{kc}")
                  for kc in range(1, CC)]
        for kc in range(1, CC):
            for c1 in range(kc):
                pt = ptp.tile([128, 128], BF16, tag="pt", name="pt")
                nc.tensor.transpose(
                    pt[:], S_sb[c1][:, (kc - c1) * 128:(kc - c1 + 1) * 128],
                    ident_bf[:])
                if (kc + c1) % 2 == 0:
                    nc.vector.tensor_copy(
                        mir_sb[kc - 1][:, c1 * 128:(c1 + 1) * 128], pt[:])
                else:
                    nc.scalar.copy(
                        mir_sb[kc - 1][:, c1 * 128:(c1 + 1) * 128], pt[:])
        U_sb = [usb.tile([128, D], BF16, tag=f"u{kc}", name=f"u{kc}")
                for kc in range(CC)]
        # U[kc] = sum_k2 S(k2,kc)^T Wk[k2] with wide (512/256) moving
        # operands; then per-pair G extraction.
        for kc in range(CC):
            up = ups.tile([128, D], F32, tag="up", name="up")
            for k2 in range(CC):
                if k2 <= kc:
                    lhsT = S_sb[k2][:, (kc - k2) * 128:(kc - k2 + 1) * 128]
                else:
                    lhsT = mir_sb[k2 - 1][:, kc * 128:(kc + 1) * 128]
                for off, w in ((0, 512), (512, 256)):
                    nc.tensor.matmul(up[:, off:off + w], lhsT,
                                     wk_sb[k2][:, off:off + w],
                                     start=(k2 == 0), stop=(k2 == CC - 1))
            if kc % 2 == 0:
                nc.vector.tensor_copy(U_sb[kc][:], up[:])
            else:
                nc.scalar.copy(U_sb[kc][:], up[:])
        with tc.tile_pool(name="gps2", bufs=2, space="PSUM") as gps2:
            for p in range(CC):
                gp = gps2.tile([128, 128], F32, tag="gp", name="gp")
                for kc in range(CC):
                    nc.tensor.matmul(gp[:], wk_sb[kc][:, p * 128:(p + 1) * 128],
                                     U_sb[kc][:, p * 128:(p + 1) * 128],
                                     start=(kc == 0), stop=(kc == CC - 1))
                for par in range(2):
                    sl = slice(par * 64, (par + 1) * 64)
                    if par == 0:
                        nc.vector.tensor_scalar_mul(
                            out=G_bf[sl, p * 64:(p + 1) * 64],
                            in0=gp[sl, sl], scalar1=GS)
                    else:
                        nc.scalar.activation(
                            G_bf[sl, p * 64:(p + 1) * 64], gp[sl, sl],
                            AF.Copy, scale=GS)

    # ---- diffusion steps ----
    for s in range(STEPS):
        # A: qv natural = x @ Wqv via xT-stationary matmuls.  Steps 1-2:
        # xT holds rsig-scaled pre-LN x; Wqv is gamma-scaled; two extra
        # outer-product rows add the -rm*gqv and bqv LayerNorm terms.
        with tc.tile_pool(name=f"qvps{s}", bufs=1, space="PSUM") as qvps:
            qv_ps = [qvps.tile([128, 2 * D], F32, tag=f"qv{n}", name=f"qv{n}")
                     for n in range(2)]
            for n in range(2):
                for kc in range(CC):
                    for m in range(3):
                        nc.tensor.matmul(
                            qv_ps[n][:, m * 512:(m + 1) * 512],
                            xT_sb[kc][:, n * 128:(n + 1) * 128],
                            Wqv_sb[kc][:, m * 512:(m + 1) * 512],
                            start=(kc == 0),
                            stop=(kc == CC - 1 and s == 0))
            if s > 0:
                for n in range(2):
                    for m in range(3):
                        nc.tensor.matmul(
                            qv_ps[n][:, m * 512:(m + 1) * 512],
                            aug_sb[:, n * 128:(n + 1) * 128],
                            gbq_sb[:, m * 512:(m + 1) * 512],
                            start=False, stop=True)
            for n in range(2):
                nc.vector.tensor_copy(q_sb[n][:], qv_ps[n][:, 0:D])
                nc.scalar.copy(v_sb[n][:], qv_ps[n][:, D:2 * D])
        if s == 0:
            # fold gamma into Wqv for the remaining steps (in place)
            for kc in range(CC):
                nc.vector.tensor_scalar_mul(
                    out=Wqv_sb[kc][:], in0=Wqv_sb[kc][:],
                    scalar1=gcol_sb[:, kc:kc + 1])

        # q^T via PE transposes
        with tc.tile_pool(name=f"qtp{s}", bufs=3, space="PSUM") as qtp:
            for cc in range(CC):
                for n in range(2):
                    pt = qtp.tile([128, 128], BF16, tag="pt", name="pt")
                    nc.tensor.transpose(pt[:], q_sb[n][:, cc * 128:(cc + 1) * 128],
                                        ident_bf[:])
                    if (cc + n) % 2 == 0:
                        nc.vector.tensor_copy(qT_sb[cc][:, n * 128:(n + 1) * 128],
                                              pt[:])
                    else:
                        nc.scalar.copy(qT_sb[cc][:, n * 128:(n + 1) * 128], pt[:])

        # attention: w = q^T v, P = G w, z^T = P^T q^T
        with tc.tile_pool(name=f"wps{s}", bufs=1, space="PSUM") as wps, \
             tc.tile_pool(name=f"pps{s}", bufs=1, space="PSUM") as pps, \
             tc.tile_pool(name=f"zps{s}", bufs=1, space="PSUM") as zps:
            W_ps = wps.tile([128, H * 32], F32, tag="w", name="w")
            for h in range(H):
                par, pr = h % 2, h // 2
                for n in range(2):
                    nc.tensor.matmul(
                        W_ps[par * 64:(par + 1) * 64, pr * 64:(pr + 1) * 64],
                        q_sb[n][:, h * 64:(h + 1) * 64],
                        v_sb[n][:, h * 64:(h + 1) * 64],
                        start=(n == 0), stop=(n == 1))
            nc.vector.tensor_copy(wb_sb[:], W_ps[:])
            P_ps = pps.tile([128, H * 32], F32, tag="p", name="p")
            for h in range(H):
                par, pr = h % 2, h // 2
                sl = slice(par * 64, (par + 1) * 64)
                cl = slice(pr * 64, (pr + 1) * 64)
                nc.tensor.matmul(P_ps[sl, cl], G_bf[sl, cl], wb_sb[sl, cl],
                                 start=True, stop=True)
            nc.scalar.copy(Pb_sb[:], P_ps[:])
            z_ps = zps.tile([128, H * 128], F32, tag="z", name="z")  # 3 banks
            for h in range(H):
                par, pr = h % 2, h // 2
                sl = slice(par * 64, (par + 1) * 64)
                nc.tensor.matmul(
                    z_ps[sl, pr * 256:(pr + 1) * 256],
                    Pb_sb[sl, pr * 64:(pr + 1) * 64],
                    qT_sb[pr][sl, :],
                    start=True, stop=True)

            # E: drain z + duplicate parity halves + strided regather
            for h in range(H):
                par, pr = h % 2, h // 2
                sl = slice(par * 64, (par + 1) * 64)
                dst = m_sb[sl, h * N:(h + 1) * N]
                src = z_ps[sl, pr * 256:pr * 256 + N]
                if par == 0:
                    nc.vector.tensor_copy(dst, src)
                else:
                    nc.scalar.copy(dst, src)
        ev = m_sb[0:64, :].rearrange("p (h n) -> p h n", n=N)[:, 0::2, :]
        ev_d = m_sb[64:128, :].rearrange("p (h n) -> p h n", n=N)[:, 0::2, :]
        od = m_sb[64:128, :].rearrange("p (h n) -> p h n", n=N)[:, 1::2, :]
        od_d = m_sb[0:64, :].rearrange("p (h n) -> p h n", n=N)[:, 1::2, :]
        nc.sync.dma_start(out=ev_d, in_=ev)
        nc.sync.dma_start(out=od_d, in_=od)
        for cc in range(CC):
            nc.vector.tensor_copy(zT_sb[cc][0:64, :], m_sb[0:64, 2 * cc::12])
            nc.scalar.copy(zT_sb[cc][64:128, :], m_sb[64:128, 2 * cc + 1::12])

        # F: xb = z @ Wproj + bproj in PSUM (ones-row K=1 matmul adds the
        # bias); LN stats straight off PSUM.
        with tc.tile_pool(name=f"fps{s}", bufs=1, space="PSUM") as fps:
            xp_ps = [fps.tile([128, D], F32, tag=f"xp{n}", name=f"xp{n}")
                     for n in range(2)]
            for n in range(2):
                for kc in range(CC):
                    for off, w in ((0, 512), (512, 256)):
                        nc.tensor.matmul(xp_ps[n][:, off:off + w],
                                         zT_sb[kc][:, n * 128:(n + 1) * 128],
                                         Wproj_sb[kc][:, off:off + w],
                                         start=(kc == 0), stop=False)
                for off, w in ((0, 512), (512, 256)):
                    nc.tensor.matmul(xp_ps[n][:, off:off + w],
                                     aug_sb[0:1, n * 128:(n + 1) * 128],
                                     bprow_sb[:, off:off + w],
                                     start=False, stop=True)
            for n in range(2):
                xv = xp_ps[n][:].rearrange("p (a b) -> p a b", b=256)
                for g in range(3):
                    nc.vector.bn_stats(out=stats[n][:, g, :], in_=xv[:, g, :])
                nc.vector.bn_aggr(out=mv[n][:], in_=stats[n][:])
                # rsig = exp(-0.5*ln(var+eps)) — both on ScalarE, avoids
                # the ~1.7us DVE reciprocal on the step-boundary path
                nc.scalar.activation(rsig[n][:], mv[n][:, 1:2], AF.Ln,
                                     bias=eps_sb[:])
                nc.scalar.activation(rsig[n][:], rsig[n][:], AF.Exp,
                                     scale=-0.5)
                if s < STEPS - 1:
                    # drain pre-LN x scaled by rsig; mean/beta terms are
                    # folded into the next A matmul via aug rows
                    nc.vector.tensor_scalar_mul(
                        out=xnat_b[n][:], in0=xp_ps[n][:],
                        scalar1=rsig[n][:, 0:1])
                    nc.vector.tensor_mul(rm_bf[n][:], mv[n][:, 0:1], rsig[n][:])
                else:
                    nc.vector.scalar_tensor_tensor(
                        out=tmp_n[n][:], in0=xp_ps[n][:], scalar=mv[n][:, 0:1],
                        in1=gamma_sb[:],
                        op0=mybir.AluOpType.subtract, op1=mybir.AluOpType.mult)
                    nc.vector.scalar_tensor_tensor(
                        out=out_sb[n][:], in0=tmp_n[n][:], scalar=rsig[n][:, 0:1],
                        in1=beta_sb[:],
                        op0=mybir.AluOpType.mult, op1=mybir.AluOpType.add)
        if s < STEPS - 1:
            with tc.tile_pool(name=f"xtp{s}", bufs=3, space="PSUM") as xtp:
                for n in range(2):
                    ptr = xtp.tile([1, 128], BF16, tag="ptr", name="ptr")
                    nc.tensor.transpose(ptr[:], rm_bf[n][:], ident_bf[:])
                    nc.scalar.copy(rmtmp[0:1, n * 128:(n + 1) * 128], ptr[:])
                nc.sync.dma_start(out=aug_sb[1:2, :], in_=rmtmp[0:1, :])
                for n in range(2):
                    for cc in range(CC):
                        pt = xtp.tile([128, 128], BF16, tag="pt", name="pt")
                        nc.tensor.transpose(
                            pt[:], xnat_b[n][:, cc * 128:(cc + 1) * 128],
                            ident_bf[:])
                        if (cc + n) % 2 == 0:
                            nc.vector.tensor_copy(
                                xT_sb[cc][:, n * 128:(n + 1) * 128], pt[:])
                        else:
                            nc.scalar.copy(
                                xT_sb[cc][:, n * 128:(n + 1) * 128], pt[:])
        else:
            for n in range(2):
                nc.sync.dma_start(out=t_out.ap()[n * 128:(n + 1) * 128, :],
                                  in_=out_sb[n][:])


def build():
    nc = bacc.Bacc("TRN2", target_bir_lowering=False, debug=False, num_devices=NB)
    t_xT = nc.declare_dram_parameter("xT", [D, N], BF16, isOutput=False)
    t_ref = nc.declare_dram_parameter("ref", [R * N, D], BF16, isOutput=False)
    t_Wqv = nc.declare_dram_parameter("Wqv", [D, 2 * D], BF16, isOutput=False)
    t_Wk = nc.declare_dram_parameter("Wk", [D, D], BF16, isOutput=False)
    t_Wproj = nc.declare_dram_parameter("Wproj", [D, D], BF16, isOutput=False)
    t_gammab = nc.declare_dram_parameter("gammab", [128, D], F32, isOutput=False)
    t_betab = nc.declare_dram_parameter("betab", [128, D], F32, isOutput=False)
    t_gcol = nc.declare_dram_parameter("gcol", [128, CC], F32, isOutput=False)
    t_gbq = nc.declare_dram_parameter("gbq", [2, 2 * D], BF16, isOutput=False)
    t_bprow = nc.declare_dram_parameter("bprow", [1, D], BF16, isOutput=False)
    t_out = nc.declare_dram_parameter("out", [N, D], F32, isOutput=True)
    with tile.TileContext(nc) as tc:
        with ExitStack() as ctx:
            _emit(nc, tc, ctx, t_xT, t_ref, t_Wqv, t_Wk, t_Wproj, t_gammab,
                  t_betab, t_gcol, t_gbq, t_bprow, t_out)
    nc.compile()
    return nc


_CACHE = {}
last_results = None


def kernel(x, ref, Wqv, Wk, Wproj, bproj, gamma, beta):
    global last_results
    if "nc" not in _CACHE:
        _CACHE["nc"] = build()
    nc = _CACHE["nc"]

    bf = ml_dtypes.bfloat16

    def f32(a):
        return np.ascontiguousarray(np.asarray(a), dtype=np.float32)

    x = f32(x)
    ref_flat = f32(ref).reshape(R * N, D)
    Wqv_f = f32(Wqv)
    gamma_f = f32(gamma)
    beta_f = f32(beta)
    bproj_f = f32(bproj)
    gbq = np.stack([beta_f @ Wqv_f, -(gamma_f @ Wqv_f)]).astype(bf)
    common = dict(
        ref=np.ascontiguousarray(ref_flat.astype(bf)),
        Wqv=np.ascontiguousarray(Wqv_f.astype(bf)),
        Wk=np.ascontiguousarray(f32(Wk).astype(bf)),
        Wproj=np.ascontiguousarray(f32(Wproj).astype(bf)),
        gammab=np.ascontiguousarray(np.broadcast_to(gamma_f, (128, D))),
        betab=np.ascontiguousarray(np.broadcast_to(beta_f, (128, D))),
        gcol=np.ascontiguousarray(gamma_f.reshape(CC, 128).T),
        gbq=np.ascontiguousarray(gbq),
        bprow=np.ascontiguousarray(bproj_f.astype(bf).reshape(1, D)),
    )
    in_maps = []
    for b in range(NB):
        xT = np.ascontiguousarray(x[b].T.astype(bf))
        in_maps.append(dict(xT=xT, **common))
    res = run_bass_kernel_spmd(nc, in_maps, list(range(NB)))
    last_results = res
    return np.stack([res.results[b]["out"] for b in range(NB)]).astype(np.float32)


# revision 15
# speedup vs baseline: 1.6946x; 1.0013x over previous
"""Trainium2 Bass kernel for nn_Diffuser (sparse_attention), v3.

Algebra: the reference attention has NO softmax, so per head
    mean_r y_rh = q_h @ Gbar_h @ (q_h^T v_h),
    Gbar_h = s^2/R * sum_r K_rh^T K_rh,  K = ref @ Wk   (64x64 per head).

v3 design:
  - bf16 matmul inputs (f32 PSUM accumulation); host pre-casts and
    pre-transposes (layout-only prep).  All matmuls 1 cycle/row + FWL.
  - Replicated Gbar precompute via triangular S = ref^T ref: only the 21
    upper tiles (one 8-bank PSUM pass over 20 streamed chunks), mirrors
    by PE transpose, U = S@Wk per pair, pair-diagonal G extraction.
    (A 196KB AllReduce alone measures ~90-110us on this 8-core setup —
    replication is strictly faster.)
  - LayerNorm folded into the next step's QV matmul: bias via K=1
    ones-row matmuls into the F PSUM, bn_stats on PSUM, drain scaled by
    rsig, and mean/beta corrections as two outer-product rows appended
    to the QV contraction (Wqv is gamma-scaled in-place on device after
    step 0).  Only the final step materializes the normalized output.
"""

import numpy as np
import ml_dtypes
from contextlib import ExitStack

import concourse.bass as bass
import concourse.tile as tile
from concourse import bacc, mybir
from concourse.bass_utils import run_bass_kernel_spmd
from concourse.masks import make_identity

F32 = mybir.dt.float32
BF16 = mybir.dt.bfloat16
AF = mybir.ActivationFunctionType

D = 768
H = 12
HD = 64
R = 10
N = 256
STEPS = 3
NB = 8
CC = D // 128  # 6
SCALE = HD ** -0.5
EPS = 1e-5
GS = SCALE * SCALE / R  # folded into Gbar
NCH = (R * N) // 128    # 20 ref chunks


def _emit(nc, tc, ctx, t_xT, t_ref, t_Wqv, t_Wk, t_Wproj, t_gammab, t_betab,
          t_gcol, t_gbq, t_bprow, t_out):
    const = ctx.enter_context(tc.tile_pool(name="const", bufs=1))
    persist = ctx.enter_context(tc.tile_pool(name="persist", bufs=1))

    eps_sb = const.tile([128, 1], F32)
    nc.vector.memset(eps_sb, EPS)
    ident_bf = const.tile([128, 128], BF16)
    make_identity(nc, ident_bf)

    # ---- input DMAs in order of first use (ref chunks feed the Gram) ----
    refp = ctx.enter_context(tc.tile_pool(name="refp", bufs=1))
    ref_ch = [refp.tile([128, D], BF16, tag=f"rc{m}", name=f"rc{m}")
              for m in range(NCH)]
    for m in range(NCH):
        nc.sync.dma_start(out=ref_ch[m], in_=t_ref.ap()[m * 128:(m + 1) * 128, :])
    wk_sb = [persist.tile([128, D], BF16, tag=f"wk{k}", name=f"wk{k}")
             for k in range(CC)]
    xT_sb = [persist.tile([128, N], BF16, tag=f"xT{k}", name=f"xT{k}")
             for k in range(CC)]
    Wqv_sb = [persist.tile([128, 2 * D], BF16, tag=f"wqv{k}", name=f"wqv{k}")
              for k in range(CC)]
    Wproj_sb = [persist.tile([128, D], BF16, tag=f"wp{k}", name=f"wp{k}")
                for k in range(CC)]
    for k in range(CC):
        nc.sync.dma_start(out=wk_sb[k], in_=t_Wk.ap()[k * 128:(k + 1) * 128, :])
    for k in range(CC):
        nc.sync.dma_start(out=xT_sb[k], in_=t_xT.ap()[k * 128:(k + 1) * 128, :])
        nc.sync.dma_start(out=Wqv_sb[k], in_=t_Wqv.ap()[k * 128:(k + 1) * 128, :])
    for k in range(CC):
        nc.sync.dma_start(out=Wproj_sb[k], in_=t_Wproj.ap()[k * 128:(k + 1) * 128, :])
    gamma_sb = const.tile([128, D], F32)
    beta_sb = const.tile([128, D], F32)
    gcol_sb = const.tile([128, CC], F32)
    gbq_sb = const.tile([2, 2 * D], BF16)
    bprow_sb = const.tile([1, D], BF16)
    nc.sync.dma_start(out=gcol_sb, in_=t_gcol.ap())
    nc.sync.dma_start(out=gbq_sb, in_=t_gbq.ap())
    nc.sync.dma_start(out=bprow_sb, in_=t_bprow.ap())
    nc.sync.dma_start(out=gamma_sb, in_=t_gammab.ap())
    nc.sync.dma_start(out=beta_sb, in_=t_betab.ap())

    # ---- persistent state ----
    q_sb = [persist.tile([128, D], BF16, tag=f"q{n}", name=f"q{n}") for n in range(2)]
    v_sb = [persist.tile([128, D], BF16, tag=f"v{n}", name=f"v{n}") for n in range(2)]
    qT_sb = [persist.tile([128, N], BF16, tag=f"qT{k}", name=f"qT{k}")
             for k in range(CC)]
    wb_sb = persist.tile([128, H * 32], BF16, tag="wb", name="wb")   # [128, 384]
    Pb_sb = persist.tile([128, H * 32], BF16, tag="Pb", name="Pb")
    G_bf = persist.tile([128, H * 32], BF16, tag="Gbf", name="Gbf")
    m_sb = persist.tile([128, H * N], BF16, tag="m_sb", name="m_sb")
    zT_sb = [persist.tile([128, N], BF16, tag=f"zT{k}", name=f"zT{k}")
             for k in range(CC)]
    tmp_n = [persist.tile([128, D], F32, tag=f"tmpn{n}", name=f"tmpn{n}")
             for n in range(2)]
    xnat_b = [persist.tile([128, D], BF16, tag=f"xn{n}", name=f"xn{n}")
              for n in range(2)]
    out_sb = [persist.tile([128, D], F32, tag=f"os{n}", name=f"os{n}")
              for n in range(2)]
    stats = [persist.tile([128, 3, 6], F32, tag=f"st{n}", name=f"st{n}")
             for n in range(2)]
    mv = [persist.tile([128, 2], F32, tag=f"mv{n}", name=f"mv{n}") for n in range(2)]
    rsig = [persist.tile([128, 1], F32, tag=f"rs{n}", name=f"rs{n}")
            for n in range(2)]
    sig_f = [persist.tile([128, 1], F32, tag=f"sg{n}", name=f"sg{n}")
             for n in range(2)]
    sigmu = [persist.tile([128, 2], BF16, tag=f"sm{n}", name=f"sm{n}")
             for n in range(2)]
    aug_sb = persist.tile([2, N], BF16, tag="aug", name="aug")
    ones_sb = persist.tile([1, N], BF16, tag="ones", name="ones")
    nc.vector.memset(ones_sb[:], 1.0)

    # ---- precompute (replicated): Gbar via triangular S = ref^T ref ----
    ssb = ctx.enter_context(tc.tile_pool(name="ssb", bufs=1))
    mirp = ctx.enter_context(tc.tile_pool(name="mirp", bufs=1))
    usb = ctx.enter_context(tc.tile_pool(name="usb", bufs=1))
    with tc.tile_pool(name="sps", bufs=1, space="PSUM") as sps:
        S_ps = [sps.tile([128, D - 128 * c1], F32, tag=f"s{c1}", name=f"s{c1}")
                for c1 in range(CC)]
        for m in range(NCH):
            for c1 in range(CC):
                wleft = D - 128 * c1
                for off in range(0, wleft, 512):
                    w = min(512, wleft - off)
                    nc.tensor.matmul(
                        S_ps[c1][:, off:off + w],
                        ref_ch[m][:, c1 * 128:(c1 + 1) * 128],
                        ref_ch[m][:, c1 * 128 + off:c1 * 128 + off + w],
                        start=(m == 0), stop=(m == NCH - 1))
        S_sb = [ssb.tile([128, D - 128 * c1], BF16, tag=f"sb{c1}", name=f"sb{c1}")
                for c1 in range(CC)]
        for c1 in range(CC):
            if c1 % 2 == 0:
                nc.vector.tensor_copy(S_sb[c1][:], S_ps[c1][:])
            else:
                nc.scalar.copy(S_sb[c1][:], S_ps[c1][:])
    with tc.tile_pool(name="ptp", bufs=2, space="PSUM") as ptp, \
         tc.tile_pool(name="ups", bufs=2, space="PSUM") as ups:
        mir_sb = [mirp.tile([128, kc * 128], BF16, tag=f"mir{kc}", name=f"mir{kc}")
                  for kc in range(1, CC)]
        for kc in range(1, CC):
            for c1 in range(kc):
                pt = ptp.tile([128, 128], BF16, tag="pt", name="pt")
                nc.tensor.transpose(
                    pt[:], S_sb[c1][:, (kc - c1) * 128:(kc - c1 + 1) * 128],
                    ident_bf[:])
                if (kc + c1) % 2 == 0:
                    nc.vector.tensor_copy(
                        mir_sb[kc - 1][:, c1 * 128:(c1 + 1) * 128], pt[:])
                else:
                    nc.scalar.copy(
                        mir_sb[kc - 1][:, c1 * 128:(c1 + 1) * 128], pt[:])
        U_sb = [usb.tile([128, D], BF16, tag=f"u{kc}", name=f"u{kc}")
                for kc in range(CC)]
        # U[kc] = sum_k2 S(k2,kc)^T Wk[k2] with wide (512/256) moving
        # operands; then per-pair G extraction.
        for kc in range(CC):
            up = ups.tile([128, D], F32, tag="up", name="up")
            for k2 in range(CC):
                if k2 <= kc:
                    lhsT = S_sb[k2][:, (kc - k2) * 128:(kc - k2 + 1) * 128]
                else:
                    lhsT = mir_sb[k2 - 1][:, kc * 128:(kc + 1) * 128]
                for off, w in ((0, 512), (512, 256)):
                    nc.tensor.matmul(up[:, off:off + w], lhsT,
                                     wk_sb[k2][:, off:off + w],
                                     start=(k2 == 0), stop=(k2 == CC - 1))
            if kc % 2 == 0:
                nc.vector.tensor_copy(U_sb[kc][:], up[:])
            else:
                nc.scalar.copy(U_sb[kc][:], up[:])
        with tc.tile_pool(name="gps2", bufs=2, space="PSUM") as gps2:
            for p in range(CC):
                gp = gps2.tile([128, 128], F32, tag="gp", name="gp")
                for kc in range(CC):
                    nc.tensor.matmul(gp[:], wk_sb[kc][:, p * 128:(p + 1) * 128],
                                     U_sb[kc][:, p * 128:(p + 1) * 128],
                                     start=(kc == 0), stop=(kc == CC - 1))
                for par in range(2):
                    sl = slice(par * 64, (par + 1) * 64)
                    if par == 0:
                        nc.vector.tensor_scalar_mul(
                            out=G_bf[sl, p * 64:(p + 1) * 64],
                            in0=gp[sl, sl], scalar1=GS)
                    else:
                        nc.scalar.activation(
                            G_bf[sl, p * 64:(p + 1) * 64], gp[sl, sl],
                            AF.Copy, scale=GS)

    # ---- diffusion steps ----
    for s in range(STEPS):
        # A: qv natural = x @ Wqv via xT-stationary matmuls.  Steps 1-2:
        # xT holds pre-LN x, Wqv is gamma-scaled, two outer-product rows
        # [sig; -mu] x [bqv; gqv] complete the fold, and the drain scales
        # by rsig (rsig*sig = 1 keeps the beta term exact).
        with tc.tile_pool(name=f"qvps{s}", bufs=1, space="PSUM") as qvps:
            qv_ps = [qvps.tile([128, 2 * D], F32, tag=f"qv{n}", name=f"qv{n}")
                     for n in range(2)]
            for n in range(2):
                for kc in range(CC):
                    for m in range(3):
                        nc.tensor.matmul(
                            qv_ps[n][:, m * 512:(m + 1) * 512],
                            xT_sb[kc][:, n * 128:(n + 1) * 128],
                            Wqv_sb[kc][:, m * 512:(m + 1) * 512],
                            start=(kc == 0),
                            stop=(kc == CC - 1 and s == 0))
            if s > 0:
                for n in range(2):
                    for m in range(3):
                        nc.tensor.matmul(
                            qv_ps[n][:, m * 512:(m + 1) * 512],
                            aug_sb[:, n * 128:(n + 1) * 128],
                            gbq_sb[:, m * 512:(m + 1) * 512],
                            start=False, stop=True)
            for n in range(2):
                if s == 0:
                    nc.vector.tensor_copy(q_sb[n][:], qv_ps[n][:, 0:D])
                    nc.scalar.copy(v_sb[n][:], qv_ps[n][:, D:2 * D])
                else:
                    nc.vector.tensor_scalar_mul(
                        out=q_sb[n][:], in0=qv_ps[n][:, 0:D],
                        scalar1=rsig[n][:, 0:1])
                    nc.vector.tensor_scalar_mul(
                        out=v_sb[n][:], in0=qv_ps[n][:, D:2 * D],
                        scalar1=rsig[n][:, 0:1])
        if s == 0:
            # fold gamma into Wqv for the remaining steps (in place)
            for kc in range(CC):
                nc.vector.tensor_scalar_mul(
                    out=Wqv_sb[kc][:], in0=Wqv_sb[kc][:],
                    scalar1=gcol_sb[:, kc:kc + 1])

        # q^T via PE transposes
        with tc.tile_pool(name=f"qtp{s}", bufs=3, space="PSUM") as qtp:
            for cc in range(CC):
                for n in range(2):
                    pt = qtp.tile([128, 128], BF16, tag="pt", name="pt")
                    nc.tensor.transpose(pt[:], q_sb[n][:, cc * 128:(cc + 1) * 128],
                                        ident_bf[:])
                    if (cc + n) % 2 == 0:
                        nc.vector.tensor_copy(qT_sb[cc][:, n * 128:(n + 1) * 128],
                                              pt[:])
                    else:
                        nc.scalar.copy(qT_sb[cc][:, n * 128:(n + 1) * 128], pt[:])

        # attention: w = q^T v, P = G w, z^T = P^T q^T
        with tc.tile_pool(name=f"wps{s}", bufs=1, space="PSUM") as wps, \
             tc.tile_pool(name=f"pps{s}", bufs=1, space="PSUM") as pps, \
             tc.tile_pool(name=f"zps{s}", bufs=1, space="PSUM") as zps:
            W_ps = wps.tile([128, H * 32], F32, tag="w", name="w")
            for h in range(H):
                par, pr = h % 2, h // 2
                for n in range(2):
                    nc.tensor.matmul(
                        W_ps[par * 64:(par + 1) * 64, pr * 64:(pr + 1) * 64],
                        q_sb[n][:, h * 64:(h + 1) * 64],
                        v_sb[n][:, h * 64:(h + 1) * 64],
                        start=(n == 0), stop=(n == 1))
            nc.vector.tensor_copy(wb_sb[:], W_ps[:])
            P_ps = pps.tile([128, H * 32], F32, tag="p", name="p")
            for h in range(H):
                par, pr = h % 2, h // 2
                sl = slice(par * 64, (par + 1) * 64)
                cl = slice(pr * 64, (pr + 1) * 64)
                nc.tensor.matmul(P_ps[sl, cl], G_bf[sl, cl], wb_sb[sl, cl],
                                 start=True, stop=True)
            nc.scalar.copy(Pb_sb[:], P_ps[:])
            z_ps = zps.tile([128, H * 128], F32, tag="z", name="z")  # 3 banks
            for h in range(H):
                par, pr = h % 2, h // 2
                sl = slice(par * 64, (par + 1) * 64)
                nc.tensor.matmul(
                    z_ps[sl, pr * 256:(pr + 1) * 256],
                    Pb_sb[sl, pr * 64:(pr + 1) * 64],
                    qT_sb[pr][sl, :],
                    start=True, stop=True)

            # E: drain z into both parity halves directly (no dup DMA),
            # then strided regather into c-major z^T
            for h in range(H):
                par, pr = h % 2, h // 2
                src = z_ps[par * 64:(par + 1) * 64, pr * 256:pr * 256 + N]
                dn = m_sb[par * 64:(par + 1) * 64, h * N:(h + 1) * N]
                dm = m_sb[(1 - par) * 64:(2 - par) * 64, h * N:(h + 1) * N]
                if par == 0:
                    nc.vector.tensor_copy(dn, src)
                    nc.scalar.copy(dm, src)
                else:
                    nc.scalar.copy(dn, src)
                    nc.vector.tensor_copy(dm, src)
        for cc in range(CC):
            nc.vector.tensor_copy(zT_sb[cc][0:64, :], m_sb[0:64, 2 * cc::12])
            nc.scalar.copy(zT_sb[cc][64:128, :], m_sb[64:128, 2 * cc + 1::12])

        # F: xb = z @ Wproj + bproj in PSUM (ones-row K=1 matmul adds the
        # bias); LN stats straight off PSUM.
        with tc.tile_pool(name=f"fps{s}", bufs=1, space="PSUM") as fps:
            xp_ps = [fps.tile([128, D], F32, tag=f"xp{n}", name=f"xp{n}")
                     for n in range(2)]
            for n in range(2):
                for kc in range(CC):
                    for off, w in ((0, 512), (512, 256)):
                        nc.tensor.matmul(xp_ps[n][:, off:off + w],
                                         zT_sb[kc][:, n * 128:(n + 1) * 128],
                                         Wproj_sb[kc][:, off:off + w],
                                         start=(kc == 0), stop=False)
                for off, w in ((0, 512), (512, 256)):
                    nc.tensor.matmul(xp_ps[n][:, off:off + w],
                                     ones_sb[0:1, n * 128:(n + 1) * 128],
                                     bprow_sb[:, off:off + w],
                                     start=False, stop=True)
            if s < STEPS - 1:
                for n in range(2):
                    nc.scalar.copy(xnat_b[n][:], xp_ps[n][:])
            for n in range(2):
                xv = xp_ps[n][:].rearrange("p (a b) -> p a b", b=256)
                for g in range(3):
                    nc.vector.bn_stats(out=stats[n][:, g, :], in_=xv[:, g, :])
                nc.vector.bn_aggr(out=mv[n][:], in_=stats[n][:])
                nc.scalar.activation(sig_f[n][:], mv[n][:, 1:2], AF.Sqrt,
                                     bias=eps_sb[:])
                if s < STEPS - 1:
                    nc.scalar.copy(sigmu[n][:, 0:1], sig_f[n][:])
                    nc.scalar.activation(sigmu[n][:, 1:2], mv[n][:, 0:1],
                                         AF.Copy, scale=-1.0)
                nc.vector.reciprocal(rsig[n][:], sig_f[n][:])
                if s == STEPS - 1:
                    nc.vector.scalar_tensor_tensor(
                        out=tmp_n[n][:], in0=xp_ps[n][:], scalar=mv[n][:, 0:1],
                        in1=gamma_sb[:],
                        op0=mybir.AluOpType.subtract, op1=mybir.AluOpType.mult)
                    nc.vector.scalar_tensor_tensor(
                        out=out_sb[n][:], in0=tmp_n[n][:], scalar=rsig[n][:, 0:1],
                        in1=beta_sb[:],
                        op0=mybir.AluOpType.mult, op1=mybir.AluOpType.add)
        if s < STEPS - 1:
            with tc.tile_pool(name=f"xtp{s}", bufs=3, space="PSUM") as xtp:
                for n in range(2):
                    for cc in range(CC):
                        pt = xtp.tile([128, 128], BF16, tag="pt", name="pt")
                        nc.tensor.transpose(
                            pt[:], xnat_b[n][:, cc * 128:(cc + 1) * 128],
                            ident_bf[:])
                        if (cc + n) % 2 == 0:
                            nc.vector.tensor_copy(
                                xT_sb[cc][:, n * 128:(n + 1) * 128], pt[:])
                        else:
                            nc.scalar.copy(
                                xT_sb[cc][:, n * 128:(n + 1) * 128], pt[:])
                for n in range(2):
                    pts = xtp.tile([2, 128], BF16, tag="pts", name="pts")
                    nc.tensor.transpose(pts[:], sigmu[n][:], ident_bf[:])
                    nc.scalar.copy(aug_sb[0:2, n * 128:(n + 1) * 128], pts[:])
        else:
            for n in range(2):
                nc.sync.dma_start(out=t_out.ap()[n * 128:(n + 1) * 128, :],
                                  in_=out_sb[n][:])


def build():
    nc = bacc.Bacc("TRN2", target_bir_lowering=False, debug=False, num_devices=NB)
    t_xT = nc.declare_dram_parameter("xT", [D, N], BF16, isOutput=False)
    t_ref = nc.declare_dram_parameter("ref", [R * N, D], BF16, isOutput=False)
    t_Wqv = nc.declare_dram_parameter("Wqv", [D, 2 * D], BF16, isOutput=False)
    t_Wk = nc.declare_dram_parameter("Wk", [D, D], BF16, isOutput=False)
    t_Wproj = nc.declare_dram_parameter("Wproj", [D, D], BF16, isOutput=False)
    t_gammab = nc.declare_dram_parameter("gammab", [128, D], F32, isOutput=False)
    t_betab = nc.declare_dram_parameter("betab", [128, D], F32, isOutput=False)
    t_gcol = nc.declare_dram_parameter("gcol", [128, CC], F32, isOutput=False)
    t_gbq = nc.declare_dram_parameter("gbq", [2, 2 * D], BF16, isOutput=False)
    t_bprow = nc.declare_dram_parameter("bprow", [1, D], BF16, isOutput=False)
    t_out = nc.declare_dram_parameter("out", [N, D], F32, isOutput=True)
    with tile.TileContext(nc) as tc:
        with ExitStack() as ctx:
            _emit(nc, tc, ctx, t_xT, t_ref, t_Wqv, t_Wk, t_Wproj, t_gammab,
                  t_betab, t_gcol, t_gbq, t_bprow, t_out)
    nc.compile()
    return nc


_CACHE = {}
last_results = None


def kernel(x, ref, Wqv, Wk, Wproj, bproj, gamma, beta):
    global last_results
    if "nc" not in _CACHE:
        _CACHE["nc"] = build()
    nc = _CACHE["nc"]

    bf = ml_dtypes.bfloat16

    def f32(a):
        return np.ascontiguousarray(np.asarray(a), dtype=np.float32)

    x = f32(x)
    ref_flat = f32(ref).reshape(R * N, D)
    Wqv_f = f32(Wqv)
    gamma_f = f32(gamma)
    beta_f = f32(beta)
    bproj_f = f32(bproj)
    gbq = np.stack([beta_f @ Wqv_f, gamma_f @ Wqv_f]).astype(bf)
    common = dict(
        ref=np.ascontiguousarray(ref_flat.astype(bf)),
        Wqv=np.ascontiguousarray(Wqv_f.astype(bf)),
        Wk=np.ascontiguousarray(f32(Wk).astype(bf)),
        Wproj=np.ascontiguousarray(f32(Wproj).astype(bf)),
        gammab=np.ascontiguousarray(np.broadcast_to(gamma_f, (128, D))),
        betab=np.ascontiguousarray(np.broadcast_to(beta_f, (128, D))),
        gcol=np.ascontiguousarray(gamma_f.reshape(CC, 128).T),
        gbq=np.ascontiguousarray(gbq),
        bprow=np.ascontiguousarray(bproj_f.astype(bf).reshape(1, D)),
    )
    in_maps = []
    for b in range(NB):
        xT = np.ascontiguousarray(x[b].T.astype(bf))
        in_maps.append(dict(xT=xT, **common))
    res = run_bass_kernel_spmd(nc, in_maps, list(range(NB)))
    last_results = res
    return np.stack([res.results[b]["out"] for b in range(NB)]).astype(np.float32)
